# Initial kernel scaffold
#
"""Trainium2 Bass kernel for nn_ExpressionModule_2267742732789.

Expression tree (DEPTH=4, preorder params, 25 scalars):
    x2 = x*x
    t1 = tanh(p7 *x2)   t2 = tanh(p8 *x2)   u1 = p4*t1 + p5*t2 + p6
    t3 = tanh(p12*x2)   t4 = tanh(p13*x2)   u2 = p9*t3 + p10*t4 + p11
    v1 = tanh(p3 * u1*u2)
    t5 = tanh(p18*x2)   t6 = tanh(p19*x2)   u3 = p15*t5 + p16*t6 + p17
    t7 = tanh(p23*x2)   t8 = tanh(p24*x2)   u4 = p20*t7 + p21*t8 + p22
    v2 = tanh(p14 * u3*u4)
    out = p0*v1 + p1*v2 + p2

Sharding: x (16M fp32) split evenly across 8 NeuronCores (data parallel);
the 25 params are baked into instruction immediates at call time (JIT
specialization -- recompiles for new param values, correct for any input).

Per-core engine split (all elementwise, ACT-bound at 10 tanh passes):
    ACT    : 8 leaf tanh (scale=p_k folded) + 2 mid tanh (scale folded)
    DVE    : x^2 (TT), 4 waff combines (scalar_tensor_tensor), 2 products
             (TT), final combine (STT)  -> 8 ops
    GPSIMD : 5 affine terms t*A + C (tensor_scalar)
"""

import os
import sys

import numpy as np

sys.path.insert(0, "/opt/trn_rl_repo")

import concourse.bass as bass
import concourse.mybir as mybir
from concourse import tile
from concourse.bass_utils import run_bass_kernel_spmd

N = 16777216
NCORES = 8
E = N // NCORES  # 2_097_152 per core
P = 128
FD = 2048
NCHUNK = E // (P * FD)  # 8

F32 = mybir.dt.float32
MULT = mybir.AluOpType.mult
ADD = mybir.AluOpType.add
TANH = mybir.ActivationFunctionType.Tanh


def build_nc(p, nchunk=NCHUNK, fd=FD):
    """Build the SPMD Bass program with params p (list of 25 floats) baked in."""
    nc = bass.Bass()
    x_h = nc.dram_tensor("x", [nchunk, P, fd], F32, kind="ExternalInput")
    o_h = nc.dram_tensor("out", [nchunk, P, fd], F32, kind="ExternalOutput")

    with tile.TileContext(nc) as tc:
        with (
            tc.tile_pool(name="io", bufs=2) as io,
            tc.tile_pool(name="wk", bufs=2) as wk,
            tc.tile_pool(name="tt", bufs=4) as tp,
            tc.tile_pool(name="uu", bufs=3) as up,
        ):
            for c in range(nchunk):
                xt = io.tile([P, fd], F32, tag="x")
                nc.sync.dma_start(out=xt[:], in_=x_h[c])
                x2 = wk.tile([P, fd], F32, tag="x2")
                nc.vector.tensor_tensor(x2[:], xt[:], xt[:], MULT)

                def half(scales, wa, wb, wc, g):
                    # returns v = tanh(g * (wa*ta + wb*tb + wc) * (...second))
                    us = []
                    for (sa, sb), (ca, cb, cc) in zip(scales, ((wa, wb, wc),) * 1):
                        pass
                    return us

                def waff_pair(s_a, s_b, w0, w1, b0):
                    ta = tp.tile([P, fd], F32, tag="t")
                    nc.scalar.activation(ta[:], x2[:], TANH, scale=s_a)
                    tb = tp.tile([P, fd], F32, tag="t")
                    nc.scalar.activation(tb[:], x2[:], TANH, scale=s_b)
                    aa = wk.tile([P, fd], F32, tag="a")
                    nc.gpsimd.tensor_scalar(aa[:], ta[:], w0, b0, MULT, ADD)
                    uu = up.tile([P, fd], F32, tag="u")
                    nc.vector.scalar_tensor_tensor(uu[:], tb[:], w1, aa[:], MULT, ADD)
                    return uu

                # left subtree -> v1
                u1 = waff_pair(p[7], p[8], p[4], p[5], p[6])
                u2 = waff_pair(p[12], p[13], p[9], p[10], p[11])
                m1 = wk.tile([P, fd], F32, tag="m")
                nc.vector.tensor_tensor(m1[:], u1[:], u2[:], MULT)
                v1 = wk.tile([P, fd], F32, tag="v")
                nc.scalar.activation(v1[:], m1[:], TANH, scale=p[3])

                # right subtree -> v2
                u3 = waff_pair(p[18], p[19], p[15], p[16], p[17])
                u4 = waff_pair(p[23], p[24], p[20], p[21], p[22])
                m2 = wk.tile([P, fd], F32, tag="m")
                nc.vector.tensor_tensor(m2[:], u3[:], u4[:], MULT)
                v2 = wk.tile([P, fd], F32, tag="v")
                nc.scalar.activation(v2[:], m2[:], TANH, scale=p[14])

                # root: out = p0*v1 + p1*v2 + p2
                cc = wk.tile([P, fd], F32, tag="c")
                nc.gpsimd.tensor_scalar(cc[:], v1[:], p[0], p[2], MULT, ADD)
                ot = io.tile([P, fd], F32, tag="o")
                nc.vector.scalar_tensor_tensor(ot[:], v2[:], p[1], cc[:], MULT, ADD)
                nc.sync.dma_start(out=o_h[c], in_=ot[:])
    return nc


_cache = {}


def kernel(x, params):
    x = np.ascontiguousarray(np.asarray(x, dtype=np.float32))
    params = np.asarray(params, dtype=np.float32)
    p = [float(v) for v in params]
    key = params.tobytes()
    if key not in _cache:
        _cache[key] = build_nc(p)
    nc = _cache[key]

    shards = x.reshape(NCORES, NCHUNK, P, FD)
    in_maps = [{"x": shards[i]} for i in range(NCORES)]
    trace = bool(int(os.environ.get("BASS_EXPR_TRACE", "0")))
    res = run_bass_kernel_spmd(nc, in_maps, list(range(NCORES)), trace=trace)
    out = np.concatenate([res.results[i]["out"].reshape(-1) for i in range(NCORES)])
    if trace:
        kernel.last_exec_time_ns = res.exec_time_ns
        kernel.last_results = res
    return out


# revision 7
# speedup vs baseline: 2.6991x; 2.6991x over previous
"""Trainium2 Bass kernel for nn_ExpressionModule_2267742732789.

Expression tree (DEPTH=4, preorder params, 25 scalars):
    x2 = x*x
    t1 = tanh(p7 *x2)   t2 = tanh(p8 *x2)   u1 = p4*t1 + p5*t2 + p6
    t3 = tanh(p12*x2)   t4 = tanh(p13*x2)   u2 = p9*t3 + p10*t4 + p11
    v1 = tanh(p3 * u1*u2)
    t5 = tanh(p18*x2)   t6 = tanh(p19*x2)   u3 = p15*t5 + p16*t6 + p17
    t7 = tanh(p23*x2)   t8 = tanh(p24*x2)   u4 = p20*t7 + p21*t8 + p22
    v2 = tanh(p14 * u3*u4)
    out = p0*v1 + p1*v2 + p2

Sharding: x (16M fp32) split evenly across 8 NeuronCores (data parallel);
the 25 params are baked into instruction immediates at call time (JIT
specialization -- recompiles for new param values, correct for any input).

Per-core engine split (all elementwise, ACT-bound at 10 tanh passes):
    ACT    : 8 leaf tanh (scale=p_k folded) + 2 mid tanh (scale folded)
    DVE    : x^2 (TT), 4 waff combines (scalar_tensor_tensor), 2 products
             (TT), final combine (STT)  -> 8 ops
    GPSIMD : 5 affine terms t*A + C (tensor_scalar)
"""

import os
import sys

import numpy as np

sys.path.insert(0, "/opt/trn_rl_repo")

import concourse.bass as bass
import concourse.bacc as bacc
import concourse.mybir as mybir
from concourse import tile
from concourse.bass_utils import run_bass_kernel_spmd

N = 16777216
NCORES = 8
E = N // NCORES  # 2_097_152 per core
P = 128
FD = 2048
NCHUNK = E // (P * FD)  # 8

F32 = mybir.dt.float32
MULT = mybir.AluOpType.mult
ADD = mybir.AluOpType.add
TANH = mybir.ActivationFunctionType.Tanh


def build_nc(p, nchunk=NCHUNK, fd=FD, passes=1):
    """Build the SPMD Bass program with params p (list of 25 floats) baked in.

    passes>1 repeats the whole computation (same in/out) for benchmarking.
    """
    nc = bacc.Bacc("TRN2", target_bir_lowering=False, debug=False)
    x_h = nc.dram_tensor("x", [nchunk, P, fd], F32, kind="ExternalInput")
    o_h = nc.dram_tensor("out", [nchunk, P, fd], F32, kind="ExternalOutput")

    with tile.TileContext(nc) as tc:
        with (
            tc.tile_pool(name="xin", bufs=nchunk) as xin,
            tc.tile_pool(name="io", bufs=2) as io,
            tc.tile_pool(name="wk", bufs=2) as wk,
            tc.tile_pool(name="tt", bufs=3) as tp,
            tc.tile_pool(name="uu", bufs=2) as up,
        ):
            for c in [c for _ in range(passes) for c in range(nchunk)]:
                xt = xin.tile([P, fd], F32, tag="x")
                nc.sync.dma_start(out=xt[:], in_=x_h[c])
                x2 = wk.tile([P, fd], F32, tag="x2")
                nc.vector.tensor_tensor(x2[:], xt[:], xt[:], MULT)

                def waff_pair(s_a, s_b, w0, w1, b0):
                    ta = tp.tile([P, fd], F32, tag="t")
                    nc.scalar.activation(ta[:], x2[:], TANH, scale=s_a)
                    tb = tp.tile([P, fd], F32, tag="t")
                    nc.scalar.activation(tb[:], x2[:], TANH, scale=s_b)
                    aa = wk.tile([P, fd], F32, tag="a")
                    nc.gpsimd.tensor_scalar(aa[:], ta[:], w0, b0, MULT, ADD)
                    uu = up.tile([P, fd], F32, tag="u")
                    nc.vector.scalar_tensor_tensor(uu[:], tb[:], w1, aa[:], MULT, ADD)
                    return uu

                # left subtree -> v1
                u1 = waff_pair(p[7], p[8], p[4], p[5], p[6])
                u2 = waff_pair(p[12], p[13], p[9], p[10], p[11])
                m1 = wk.tile([P, fd], F32, tag="m")
                nc.vector.tensor_tensor(m1[:], u1[:], u2[:], MULT)
                v1 = wk.tile([P, fd], F32, tag="v")
                nc.scalar.activation(v1[:], m1[:], TANH, scale=p[3])

                # right subtree -> v2
                u3 = waff_pair(p[18], p[19], p[15], p[16], p[17])
                u4 = waff_pair(p[23], p[24], p[20], p[21], p[22])
                m2 = wk.tile([P, fd], F32, tag="m")
                nc.vector.tensor_tensor(m2[:], u3[:], u4[:], MULT)
                v2 = wk.tile([P, fd], F32, tag="v")
                nc.scalar.activation(v2[:], m2[:], TANH, scale=p[14])

                # root: out = p0*v1 + p1*v2 + p2
                cc = wk.tile([P, fd], F32, tag="a")
                nc.gpsimd.tensor_scalar(cc[:], v1[:], p[0], p[2], MULT, ADD)
                ot = io.tile([P, fd], F32, tag="o")
                nc.vector.scalar_tensor_tensor(ot[:], v2[:], p[1], cc[:], MULT, ADD)
                nc.sync.dma_start(out=o_h[c], in_=ot[:])
    nc.compile()
    return nc


_cache = {}


def kernel(x, params):
    x = np.ascontiguousarray(np.asarray(x, dtype=np.float32))
    params = np.asarray(params, dtype=np.float32)
    p = [float(v) for v in params]
    key = params.tobytes()
    if key not in _cache:
        _cache[key] = build_nc(p)
    nc = _cache[key]

    shards = x.reshape(NCORES, NCHUNK, P, FD)
    in_maps = [{"x": shards[i]} for i in range(NCORES)]
    trace = bool(int(os.environ.get("BASS_EXPR_TRACE", "0")))
    res = run_bass_kernel_spmd(nc, in_maps, list(range(NCORES)), trace=trace)
    out = np.concatenate([res.results[i]["out"].reshape(-1) for i in range(NCORES)])
    if trace:
        kernel.last_exec_time_ns = res.exec_time_ns
        kernel.last_results = res
    return out
